# revision 11
# baseline (speedup 1.0000x reference)
"""Two-scale patch attention (nn_Atten_module_86938728006215) on 8 TRN2 cores.

Sharding: core k handles batch b = k//4, query-row chunk mc = k%4 for BOTH
patch scales (scale1 = 4x4 patches -> N=4096, D=2048, 1024 rows/core;
scale0 = 8x8 patches -> N=1024, D=8192, 256 rows/core).  Every core does an
identical amount of work (~2.15e10 MACs) and needs no collectives.

The _to_tokens permutation is applied on the host during sharding, so every
device DMA is a contiguous stream:
  qT  [D, M]           feature-major query block, pre-scaled by 1/sqrt(D), f16
  kh  [NS, 128, DT, 128]  K^T n-strips:  kh[s, p, dt, n] = K^T[dt*128+p, s*128+n]
  vh  [DT, 128, NS, 128]  V d'-strips:   vh[dp, p, ns, j] = V[ns*128+p, dp*128+j]
Device computes S^T = K Q^T (so softmax's sum is a ones-matmul over the
partition axis), exp on ScalarE (no max subtraction needed: scores ~ N(0,1)),
O^T = V expS^T, then a broadcast-multiply by 1/rowsum.  Output oT [D, M] f32.
"""

import os
import numpy as np

import concourse.bass as bass
import concourse.mybir as mybir
import concourse.tile as tile
from concourse.bass_utils import run_bass_kernel_spmd

F16 = mybir.dt.float16
F32 = mybir.dt.float32
EXP = mybir.ActivationFunctionType.Exp

B, T, C, H, W = 2, 16, 256, 64, 64
# (ph, pw, N, D, M_per_core, chan_lo)
SCALE1 = (4, 4, 4096, 2048, 1024, 128)
SCALE0 = (8, 8, 1024, 8192, 256, 0)

LAST_EXEC_NS = None
LAST_RESULTS = None
_NC_CACHE = None


def _emit_job(tc, pools, qdram, kdram, vdram, odram, D, N, mo, M, vg):
    """One attention job: query columns [mo, mo+M) of a [D, N] problem.

    vg = number of d'-tiles fetched per V DMA (bigger -> fewer, larger DMAs).
    """
    nc = tc.nc
    DT, NS = D // 128, N // 128
    qpool, epool, kpool, vpool, spool, opool, pa, ps, pb, pc, ones_p, ones_1 = pools
    tagsfx = f"d{D}"

    qr = qdram[:, :].rearrange("(dt p) m -> p dt m", p=128)
    orr = odram[:, :].rearrange("(dt p) m -> p dt m", p=128)

    q_sb = qpool.tile([128, DT, M], F16, tag="q")
    nc.sync.dma_start(out=q_sb, in_=qr[:, :, mo : mo + M])

    expS = epool.tile([128, NS, M], F16, tag="e")

    # ---- Phase A: S^T strips + exp ----
    for ns in range(NS):
        k_sb = kpool.tile([128, DT * 128], F16, tag="k" + tagsfx)
        nc.sync.dma_start(out=k_sb, in_=kdram[ns].rearrange("p a b -> p (a b)"))
        psA = pa.tile([128, M], F32, tag="pa")
        for dt in range(DT):
            nc.tensor.matmul(
                psA,
                k_sb[:, dt * 128 : (dt + 1) * 128],
                q_sb[:, dt, :],
                start=(dt == 0),
                stop=(dt == DT - 1),
            )
        nc.scalar.activation(expS[:, ns, :], psA, EXP)

    # ---- softmax denominators: column sums of expS via ones-matmul ----
    psS = ps.tile([1, M], F32, tag="ps")
    for ns in range(NS):
        nc.tensor.matmul(
            psS, ones_p, expS[:, ns, :], start=(ns == 0), stop=(ns == NS - 1)
        )
    recip = spool.tile([1, M], F32, tag="recip", bufs=2)
    nc.vector.reciprocal(recip, psS)

    # ---- Phase C: O^T = V expS^T, normalized ----
    # The 1/rowsum broadcast matmul is emitted after the first C-group's
    # matmuls so the PE never stalls waiting on the DVE reciprocal.
    bc = None
    vr = vdram[:, :, :, :].rearrange("q p n j -> p q (n j)")
    for dp0 in range(0, DT, vg):
        v_sb = vpool.tile([128, vg, NS * 128], F16, tag="v" + tagsfx)
        nc.sync.dma_start(out=v_sb, in_=vr[:, dp0 : dp0 + vg, :])
        for dq in range(vg):
            psC = pc.tile([128, M], F32, tag="pc")
            for ns in range(NS):
                nc.tensor.matmul(
                    psC,
                    v_sb[:, dq, ns * 128 : (ns + 1) * 128],
                    expS[:, ns, :],
                    start=(ns == 0),
                    stop=(ns == NS - 1),
                )
            if bc is None:
                psB = pb.tile([128, M], F32, tag="pb")
                nc.tensor.matmul(psB, ones_1, recip, start=True, stop=True)
                bc = spool.tile([128, M], F16, tag="bc", bufs=2)
                nc.vector.tensor_copy(out=bc, in_=psB)
            o_sb = opool.tile([128, M], F32, tag="o")
            nc.vector.tensor_mul(o_sb, psC, bc)
            nc.scalar.dma_start(out=orr[:, dp0 + dq, mo : mo + M], in_=o_sb)


def _split_multi_waits(nc):
    """This image's walrus accepts at most ONE sem wait per instruction
    (setupSyncWait: 'Too many sync wait commands').  Tile emits joins with
    2+ waits (its transitive-minimality pass is disabled), so hoist all but
    the last wait of every instruction into 1-wait NoOps emitted just
    before it on the same engine stream."""
    nid = [0]

    def noop(engine, wait):
        nid[0] += 1
        return mybir.InstNoOp(
            name=f"waitsplit-{nid[0]}",
            ins=[],
            outs=[],
            engine=engine,
            sync_info=mybir.SyncInfo(on_wait=[wait], on_update=[]),
        )

    for f in nc.m.functions:
        for blk in f.blocks:
            insts = blk.instructions
            out = []
            changed = False
            for inst in insts:
                si = getattr(inst, "sync_info", None)
                waits = list(si.on_wait) if si is not None and si.on_wait else []
                if len(waits) > 1:
                    changed = True
                    for w in waits[:-1]:
                        out.append(noop(inst.engine, w))
                    inst.sync_info = mybir.SyncInfo(
                        on_wait=[waits[-1]], on_update=list(si.on_update)
                    )
                out.append(inst)
            if changed:
                insts[:] = out


def _build_nc():
    nc = bass.Bass("TRN2")
    q1 = nc.dram_tensor("q1", [2048, 1024], F16, kind="ExternalInput")
    k1 = nc.dram_tensor("k1", [32, 128, 16, 128], F16, kind="ExternalInput")
    v1 = nc.dram_tensor("v1", [16, 128, 32, 128], F16, kind="ExternalInput")
    q0 = nc.dram_tensor("q0", [8192, 256], F16, kind="ExternalInput")
    k0 = nc.dram_tensor("k0", [8, 128, 64, 128], F16, kind="ExternalInput")
    v0 = nc.dram_tensor("v0", [64, 128, 8, 128], F16, kind="ExternalInput")
    o1 = nc.dram_tensor("o1", [2048, 1024], F32, kind="ExternalOutput")
    o0 = nc.dram_tensor("o0", [8192, 256], F32, kind="ExternalOutput")

    with tile.TileContext(nc) as tc:
        with (
            tc.tile_pool(name="qpool", bufs=1) as qpool,
            tc.tile_pool(name="epool", bufs=2) as epool,
            tc.tile_pool(name="kpool", bufs=2) as kpool,
            tc.tile_pool(name="vpool", bufs=2) as vpool,
            tc.tile_pool(name="spool", bufs=1) as spool,
            tc.tile_pool(name="opool", bufs=2) as opool,
            tc.tile_pool(name="pa", bufs=3, space="PSUM") as pa,
            tc.tile_pool(name="ps", bufs=1, space="PSUM") as ps,
            tc.tile_pool(name="pb", bufs=1, space="PSUM") as pb,
            tc.tile_pool(name="pc", bufs=3, space="PSUM") as pc,
        ):
            ones_p = spool.tile([128, 1], F16, tag="ones_p")
            nc.vector.memset(ones_p, 1.0)
            ones_1 = spool.tile([1, 128], F32, tag="ones_1")
            nc.vector.memset(ones_1, 1.0)
            pools = (qpool, epool, kpool, vpool, spool, opool, pa, ps, pb, pc, ones_p, ones_1)
            _emit_job(tc, pools, q1, k1, v1, o1, 2048, 4096, 0, 512, vg=1)
            _emit_job(tc, pools, q1, k1, v1, o1, 2048, 4096, 512, 512, vg=1)
            _emit_job(tc, pools, q0, k0, v0, o0, 8192, 1024, 0, 256, vg=8)
    _split_multi_waits(nc)
    return nc


def _tokens(x, ph, pw):
    """(16, 128, 64, 64) -> (N, D) with n=(t,oh,ow), d=(c,ih,iw)."""
    t = x.reshape(16, 128, 64 // ph, ph, 64 // pw, pw)
    t = t.transpose(0, 2, 4, 1, 3, 5)
    return np.ascontiguousarray(t.reshape(16 * (64 // ph) * (64 // pw), 128 * ph * pw))


def _untokens(v, ph, pw):
    """(N, D) -> (16, 128, 64, 64)."""
    oh = 64 // ph
    t = v.reshape(16, oh, oh, 128, ph, pw)
    t = t.transpose(0, 3, 1, 4, 2, 5)
    return t.reshape(16, 128, 64, 64)


def _pack_scale(q, k, v, b, ph, pw, N, D, M, chan_lo):
    """Host-side shard prep for one (batch, scale): returns dict pieces."""
    sl = np.s_[b * 16 : (b + 1) * 16, chan_lo : chan_lo + 128]
    qt = _tokens(q[sl], ph, pw)  # [N, D]
    kt = _tokens(k[sl], ph, pw)
    vt = _tokens(v[sl], ph, pw)
    scale = np.float32(1.0 / np.sqrt(D))
    NS, DT = N // 128, D // 128
    # q^T per m-chunk, pre-scaled:
    qT = np.ascontiguousarray((qt.T * scale).astype(np.float16))  # [D, N]
    kh = np.ascontiguousarray(
        kt.T.reshape(DT, 128, NS, 128).transpose(2, 1, 0, 3).astype(np.float16)
    )  # [NS, 128, DT, 128]
    vh = np.ascontiguousarray(
        vt.reshape(NS, 128, DT, 128).transpose(2, 1, 0, 3).astype(np.float16)
    )  # [DT, 128, NS, 128]
    return qT, kh, vh


def kernel(query, key, value, masks):
    global LAST_EXEC_NS, LAST_RESULTS, _NC_CACHE
    query = np.asarray(query)
    key = np.asarray(key)
    value = np.asarray(value)

    if _NC_CACHE is None:
        _NC_CACHE = _build_nc()
    nc = _NC_CACHE

    in_maps = []
    packs = {}
    for b in range(B):
        packs[(b, 1)] = _pack_scale(query, key, value, b, *SCALE1)
        packs[(b, 0)] = _pack_scale(query, key, value, b, *SCALE0)
    for core in range(8):
        b, mc = core // 4, core % 4
        q1T, k1h, v1h = packs[(b, 1)]
        q0T, k0h, v0h = packs[(b, 0)]
        in_maps.append(
            {
                "q1": np.ascontiguousarray(q1T[:, mc * 1024 : (mc + 1) * 1024]),
                "k1": k1h,
                "v1": v1h,
                "q0": np.ascontiguousarray(q0T[:, mc * 256 : (mc + 1) * 256]),
                "k0": k0h,
                "v0": v0h,
            }
        )

    trace = bool(int(os.environ.get("BASS_ATTN_TRACE", "0")))
    tmpdir = os.environ.get("BASS_ATTN_TMPDIR") or None
    res = run_bass_kernel_spmd(
        nc, in_maps, core_ids=list(range(8)), trace=trace, tmpdir=tmpdir
    )
    LAST_RESULTS = res
    LAST_EXEC_NS = res.exec_time_ns

    out = np.empty((B * T, C, H, W), dtype=np.float32)
    for b in range(B):
        val1 = np.empty((4096, 2048), dtype=np.float32)
        val0 = np.empty((1024, 8192), dtype=np.float32)
        for mc in range(4):
            r = res.results[b * 4 + mc]
            val1[mc * 1024 : (mc + 1) * 1024] = r["o1"].T
            val0[mc * 256 : (mc + 1) * 256] = r["o0"].T
        out[b * 16 : (b + 1) * 16, 0:128] = _untokens(val0, 8, 8)
        out[b * 16 : (b + 1) * 16, 128:256] = _untokens(val1, 4, 4)
    return out


# revision 12
# speedup vs baseline: 1.1035x; 1.1035x over previous
"""Two-scale patch attention (nn_Atten_module_86938728006215) on 8 TRN2 cores.

Sharding: core k handles batch b = k//4, query-row chunk mc = k%4 for BOTH
patch scales (scale1 = 4x4 patches -> N=4096, D=2048, 1024 rows/core;
scale0 = 8x8 patches -> N=1024, D=8192, 256 rows/core).  Every core does an
identical amount of work (~2.15e10 MACs) and needs no collectives.

The _to_tokens permutation is applied on the host during sharding, so every
device DMA is a contiguous stream:
  qT  [D, M]           feature-major query block, pre-scaled by 1/sqrt(D), f16
  kh  [NS, 128, DT, 128]  K^T n-strips:  kh[s, p, dt, n] = K^T[dt*128+p, s*128+n]
  vh  [DT, 128, NS, 128]  V d'-strips:   vh[dp, p, ns, j] = V[ns*128+p, dp*128+j]
Device computes S^T = K Q^T (so softmax's sum is a ones-matmul over the
partition axis), exp on ScalarE (no max subtraction needed: scores ~ N(0,1)),
O^T = V expS^T, then a broadcast-multiply by 1/rowsum.  Output oT [D, M] f32.
"""

import os
import numpy as np

import concourse.bass as bass
import concourse.mybir as mybir
import concourse.tile as tile
from concourse.bass_utils import run_bass_kernel_spmd

F16 = mybir.dt.float16
F32 = mybir.dt.float32
EXP = mybir.ActivationFunctionType.Exp

B, T, C, H, W = 2, 16, 256, 64, 64
# (ph, pw, N, D, M_per_core, chan_lo)
SCALE1 = (4, 4, 4096, 2048, 1024, 128)
SCALE0 = (8, 8, 1024, 8192, 256, 0)

LAST_EXEC_NS = None
LAST_RESULTS = None
_NC_CACHE = None


def _emit_job(tc, pools, qdram, kdram, vdram, odram, D, N, mo, M, vg):
    """One attention job: query columns [mo, mo+M) of a [D, N] problem.

    vg = number of d'-tiles fetched per V DMA (bigger -> fewer, larger DMAs).
    """
    nc = tc.nc
    DT, NS = D // 128, N // 128
    qpool, epool, kpool, vpool, spool, opool, pa, ps, pb, pc, ones_p, ones_1 = pools
    tagsfx = f"d{D}"

    qr = qdram[:, :].rearrange("(dt p) m -> p dt m", p=128)
    orr = odram[:, :].rearrange("(dt p) m -> p dt m", p=128)

    q_sb = qpool.tile([128, DT, M], F16, tag="q")
    nc.sync.dma_start(out=q_sb, in_=qr[:, :, mo : mo + M])

    expS = epool.tile([128, NS, M], F16, tag="e")

    # ---- Phase A: S^T strips + exp ----
    for ns in range(NS):
        k_sb = kpool.tile(
            [128, DT * 128], F16, tag="k" + tagsfx, bufs=3 if D == 2048 else 2
        )
        nc.sync.dma_start(out=k_sb, in_=kdram[ns].rearrange("p a b -> p (a b)"))
        psA = pa.tile([128, M], F32, tag="pa")
        for dt in range(DT):
            nc.tensor.matmul(
                psA,
                k_sb[:, dt * 128 : (dt + 1) * 128],
                q_sb[:, dt, :],
                start=(dt == 0),
                stop=(dt == DT - 1),
            )
        nc.scalar.activation(expS[:, ns, :], psA, EXP)

    # ---- softmax denominators: column sums of expS via ones-matmul ----
    psS = ps.tile([1, M], F32, tag="ps")
    for ns in range(NS):
        nc.tensor.matmul(
            psS, ones_p, expS[:, ns, :], start=(ns == 0), stop=(ns == NS - 1)
        )
    recip = spool.tile([1, M], F32, tag="recip", bufs=2)
    nc.vector.reciprocal(recip, psS)

    # ---- Phase C: O^T = V expS^T, normalized ----
    # The 1/rowsum broadcast matmul is emitted after the first C-group's
    # matmuls so the PE never stalls waiting on the DVE reciprocal.
    bc = None
    vr = vdram[:, :, :, :].rearrange("q p n j -> p q (n j)")
    for dp0 in range(0, DT, vg):
        v_sb = vpool.tile([128, vg, NS * 128], F16, tag="v" + tagsfx)
        nc.sync.dma_start(out=v_sb, in_=vr[:, dp0 : dp0 + vg, :])
        for dq in range(vg):
            psC = pc.tile([128, M], F32, tag="pc")
            for ns in range(NS):
                nc.tensor.matmul(
                    psC,
                    v_sb[:, dq, ns * 128 : (ns + 1) * 128],
                    expS[:, ns, :],
                    start=(ns == 0),
                    stop=(ns == NS - 1),
                )
            if bc is None:
                psB = pb.tile([128, M], F32, tag="pb")
                nc.tensor.matmul(psB, ones_1, recip, start=True, stop=True)
                bc = spool.tile([128, M], F16, tag="bc", bufs=2)
                nc.vector.tensor_copy(out=bc, in_=psB)
            o_sb = opool.tile([128, M], F32, tag="o")
            nc.vector.tensor_mul(o_sb, psC, bc)
            nc.scalar.dma_start(out=orr[:, dp0 + dq, mo : mo + M], in_=o_sb)


def _split_multi_waits(nc):
    """This image's walrus accepts at most ONE sem wait per instruction
    (setupSyncWait: 'Too many sync wait commands').  Tile emits joins with
    2+ waits (its transitive-minimality pass is disabled), so hoist all but
    the last wait of every instruction into 1-wait NoOps emitted just
    before it on the same engine stream."""
    nid = [0]

    def noop(engine, wait):
        nid[0] += 1
        return mybir.InstNoOp(
            name=f"waitsplit-{nid[0]}",
            ins=[],
            outs=[],
            engine=engine,
            sync_info=mybir.SyncInfo(on_wait=[wait], on_update=[]),
        )

    for f in nc.m.functions:
        for blk in f.blocks:
            insts = blk.instructions
            out = []
            changed = False
            for inst in insts:
                si = getattr(inst, "sync_info", None)
                waits = list(si.on_wait) if si is not None and si.on_wait else []
                if len(waits) > 1:
                    changed = True
                    for w in waits[:-1]:
                        out.append(noop(inst.engine, w))
                    inst.sync_info = mybir.SyncInfo(
                        on_wait=[waits[-1]], on_update=list(si.on_update)
                    )
                out.append(inst)
            if changed:
                insts[:] = out


def _build_nc():
    nc = bass.Bass("TRN2")
    q1 = nc.dram_tensor("q1", [2048, 1024], F16, kind="ExternalInput")
    k1 = nc.dram_tensor("k1", [32, 128, 16, 128], F16, kind="ExternalInput")
    v1 = nc.dram_tensor("v1", [16, 128, 32, 128], F16, kind="ExternalInput")
    q0 = nc.dram_tensor("q0", [8192, 256], F16, kind="ExternalInput")
    k0 = nc.dram_tensor("k0", [8, 128, 64, 128], F16, kind="ExternalInput")
    v0 = nc.dram_tensor("v0", [64, 128, 8, 128], F16, kind="ExternalInput")
    o1 = nc.dram_tensor("o1", [2048, 1024], F32, kind="ExternalOutput")
    o0 = nc.dram_tensor("o0", [8192, 256], F32, kind="ExternalOutput")

    with tile.TileContext(nc) as tc:
        with (
            tc.tile_pool(name="qpool", bufs=1) as qpool,
            tc.tile_pool(name="epool", bufs=2) as epool,
            tc.tile_pool(name="kpool", bufs=2) as kpool,
            tc.tile_pool(name="vpool", bufs=2) as vpool,
            tc.tile_pool(name="spool", bufs=1) as spool,
            tc.tile_pool(name="opool", bufs=4) as opool,
            tc.tile_pool(name="pa", bufs=3, space="PSUM") as pa,
            tc.tile_pool(name="ps", bufs=1, space="PSUM") as ps,
            tc.tile_pool(name="pb", bufs=1, space="PSUM") as pb,
            tc.tile_pool(name="pc", bufs=3, space="PSUM") as pc,
        ):
            ones_p = spool.tile([128, 1], F16, tag="ones_p")
            nc.vector.memset(ones_p, 1.0)
            ones_1 = spool.tile([1, 128], F32, tag="ones_1")
            nc.vector.memset(ones_1, 1.0)
            pools = (qpool, epool, kpool, vpool, spool, opool, pa, ps, pb, pc, ones_p, ones_1)
            _emit_job(tc, pools, q1, k1, v1, o1, 2048, 4096, 0, 512, vg=1)
            _emit_job(tc, pools, q1, k1, v1, o1, 2048, 4096, 512, 512, vg=1)
            _emit_job(tc, pools, q0, k0, v0, o0, 8192, 1024, 0, 256, vg=8)
    _split_multi_waits(nc)
    return nc


def _tokens(x, ph, pw):
    """(16, 128, 64, 64) -> (N, D) with n=(t,oh,ow), d=(c,ih,iw)."""
    t = x.reshape(16, 128, 64 // ph, ph, 64 // pw, pw)
    t = t.transpose(0, 2, 4, 1, 3, 5)
    return np.ascontiguousarray(t.reshape(16 * (64 // ph) * (64 // pw), 128 * ph * pw))


def _untokens(v, ph, pw):
    """(N, D) -> (16, 128, 64, 64)."""
    oh = 64 // ph
    t = v.reshape(16, oh, oh, 128, ph, pw)
    t = t.transpose(0, 3, 1, 4, 2, 5)
    return t.reshape(16, 128, 64, 64)


def _pack_scale(q, k, v, b, ph, pw, N, D, M, chan_lo):
    """Host-side shard prep for one (batch, scale): returns dict pieces."""
    sl = np.s_[b * 16 : (b + 1) * 16, chan_lo : chan_lo + 128]
    qt = _tokens(q[sl], ph, pw)  # [N, D]
    kt = _tokens(k[sl], ph, pw)
    vt = _tokens(v[sl], ph, pw)
    scale = np.float32(1.0 / np.sqrt(D))
    NS, DT = N // 128, D // 128
    # q^T per m-chunk, pre-scaled:
    qT = np.ascontiguousarray((qt.T * scale).astype(np.float16))  # [D, N]
    kh = np.ascontiguousarray(
        kt.T.reshape(DT, 128, NS, 128).transpose(2, 1, 0, 3).astype(np.float16)
    )  # [NS, 128, DT, 128]
    vh = np.ascontiguousarray(
        vt.reshape(NS, 128, DT, 128).transpose(2, 1, 0, 3).astype(np.float16)
    )  # [DT, 128, NS, 128]
    return qT, kh, vh


def kernel(query, key, value, masks):
    global LAST_EXEC_NS, LAST_RESULTS, _NC_CACHE
    query = np.asarray(query)
    key = np.asarray(key)
    value = np.asarray(value)

    if _NC_CACHE is None:
        _NC_CACHE = _build_nc()
    nc = _NC_CACHE

    in_maps = []
    packs = {}
    for b in range(B):
        packs[(b, 1)] = _pack_scale(query, key, value, b, *SCALE1)
        packs[(b, 0)] = _pack_scale(query, key, value, b, *SCALE0)
    for core in range(8):
        b, mc = core // 4, core % 4
        q1T, k1h, v1h = packs[(b, 1)]
        q0T, k0h, v0h = packs[(b, 0)]
        in_maps.append(
            {
                "q1": np.ascontiguousarray(q1T[:, mc * 1024 : (mc + 1) * 1024]),
                "k1": k1h,
                "v1": v1h,
                "q0": np.ascontiguousarray(q0T[:, mc * 256 : (mc + 1) * 256]),
                "k0": k0h,
                "v0": v0h,
            }
        )

    trace = bool(int(os.environ.get("BASS_ATTN_TRACE", "0")))
    tmpdir = os.environ.get("BASS_ATTN_TMPDIR") or None
    res = run_bass_kernel_spmd(
        nc, in_maps, core_ids=list(range(8)), trace=trace, tmpdir=tmpdir
    )
    LAST_RESULTS = res
    LAST_EXEC_NS = res.exec_time_ns

    out = np.empty((B * T, C, H, W), dtype=np.float32)
    for b in range(B):
        val1 = np.empty((4096, 2048), dtype=np.float32)
        val0 = np.empty((1024, 8192), dtype=np.float32)
        for mc in range(4):
            r = res.results[b * 4 + mc]
            val1[mc * 1024 : (mc + 1) * 1024] = r["o1"].T
            val0[mc * 256 : (mc + 1) * 256] = r["o0"].T
        out[b * 16 : (b + 1) * 16, 0:128] = _untokens(val0, 8, 8)
        out[b * 16 : (b + 1) * 16, 128:256] = _untokens(val1, 4, 4)
    return out


# revision 13
# speedup vs baseline: 1.1165x; 1.0118x over previous
"""Two-scale patch attention (nn_Atten_module_86938728006215) on 8 TRN2 cores.

Sharding: core k handles batch b = k//4, query-row chunk mc = k%4 for BOTH
patch scales (scale1 = 4x4 patches -> N=4096, D=2048, 1024 rows/core;
scale0 = 8x8 patches -> N=1024, D=8192, 256 rows/core).  Every core does an
identical amount of work (~2.15e10 MACs) and needs no collectives.

The _to_tokens permutation is applied on the host during sharding, so every
device DMA is a contiguous stream:
  qT  [D, M]           feature-major query block, pre-scaled by 1/sqrt(D), f16
  kh  [NS, 128, DT, 128]  K^T n-strips:  kh[s, p, dt, n] = K^T[dt*128+p, s*128+n]
  vh  [DT, 128, NS, 128]  V d'-strips:   vh[dp, p, ns, j] = V[ns*128+p, dp*128+j]
Device computes S^T = K Q^T (so softmax's sum is a ones-matmul over the
partition axis), exp on ScalarE (no max subtraction needed: scores ~ N(0,1)),
O^T = V expS^T, then a broadcast-multiply by 1/rowsum.  Output oT [D, M] f32.
"""

import os
import numpy as np

import concourse.bass as bass
import concourse.mybir as mybir
import concourse.tile as tile
from concourse.bass_utils import run_bass_kernel_spmd

F16 = mybir.dt.float16
F32 = mybir.dt.float32
EXP = mybir.ActivationFunctionType.Exp

B, T, C, H, W = 2, 16, 256, 64, 64
# (ph, pw, N, D, M_per_core, chan_lo)
SCALE1 = (4, 4, 4096, 2048, 1024, 128)
SCALE0 = (8, 8, 1024, 8192, 256, 0)

LAST_EXEC_NS = None
LAST_RESULTS = None
_NC_CACHE = None


def _emit_job(tc, pools, qdram, kdram, vdram, odram, D, N, mo, M, vg):
    """One attention job: query columns [mo, mo+M) of a [D, N] problem.

    vg = number of d'-tiles fetched per V DMA (bigger -> fewer, larger DMAs).
    """
    nc = tc.nc
    DT, NS = D // 128, N // 128
    vdma = nc.sync.dma_start if D == 2048 else nc.scalar.dma_start
    odma = nc.scalar.dma_start if D == 2048 else nc.sync.dma_start
    qpool, epool, kpool, vpool, spool, opool, pa, ps, pb, pc, ones_p, ones_1 = pools
    tagsfx = f"d{D}"

    qr = qdram[:, :].rearrange("(dt p) m -> p dt m", p=128)
    orr = odram[:, :].rearrange("(dt p) m -> p dt m", p=128)

    def load_k(ns):
        if D == 2048:
            k_sb = kpool.tile([128, DT * 128], F16, tag="k" + tagsfx, bufs=3)
            nc.sync.dma_start(out=k_sb, in_=kdram[ns].rearrange("p a b -> p (a b)"))
            return (k_sb,)
        h = DT // 2
        src2 = kdram[ns].rearrange("p a b -> p (a b)")
        ka = kpool.tile([128, h * 128], F16, tag="ka" + tagsfx, bufs=2)
        nc.sync.dma_start(out=ka, in_=src2[:, : h * 128])
        kb = kpool.tile([128, h * 128], F16, tag="kb" + tagsfx, bufs=2)
        nc.sync.dma_start(out=kb, in_=src2[:, h * 128 :])
        return (ka, kb)

    def k_slice(parts, dt):
        h = DT // len(parts)
        return parts[dt // h][:, (dt % h) * 128 : (dt % h + 1) * 128]

    kparts0 = load_k(0)
    q_sb = qpool.tile([128, DT, M], F16, tag="q")
    nc.sync.dma_start(out=q_sb, in_=qr[:, :, mo : mo + M])

    expS = epool.tile([128, NS, M], F16, tag="e")

    # ---- Phase A: S^T strips + exp ----
    for ns in range(NS):
        kparts = kparts0 if ns == 0 else load_k(ns)
        psA = pa.tile([128, M], F32, tag="pa")
        for dt in range(DT):
            nc.tensor.matmul(
                psA,
                k_slice(kparts, dt),
                q_sb[:, dt, :],
                start=(dt == 0),
                stop=(dt == DT - 1),
            )
        nc.scalar.activation(expS[:, ns, :], psA, EXP)

    # ---- softmax denominators: column sums of expS via ones-matmul ----
    psS = ps.tile([1, M], F32, tag="ps")
    for ns in range(NS):
        nc.tensor.matmul(
            psS, ones_p, expS[:, ns, :], start=(ns == 0), stop=(ns == NS - 1)
        )
    recip = spool.tile([1, M], F32, tag="recip", bufs=2)
    nc.vector.reciprocal(recip, psS)

    # ---- Phase C: O^T = V expS^T, normalized ----
    # The 1/rowsum broadcast matmul is emitted after the first C-group's
    # matmuls so the PE never stalls waiting on the DVE reciprocal.
    bc = None
    vr = vdram[:, :, :, :].rearrange("q p n j -> p q (n j)")
    for dp0 in range(0, DT, vg):
        v_sb = vpool.tile([128, vg, NS * 128], F16, tag="v" + tagsfx)
        vdma(out=v_sb, in_=vr[:, dp0 : dp0 + vg, :])
        for dq in range(vg):
            psC = pc.tile([128, M], F32, tag="pc")
            for ns in range(NS):
                nc.tensor.matmul(
                    psC,
                    v_sb[:, dq, ns * 128 : (ns + 1) * 128],
                    expS[:, ns, :],
                    start=(ns == 0),
                    stop=(ns == NS - 1),
                )
            if bc is None:
                psB = pb.tile([128, M], F32, tag="pb")
                nc.tensor.matmul(psB, ones_1, recip, start=True, stop=True)
                bc = spool.tile([128, M], F16, tag="bc", bufs=2)
                nc.vector.tensor_copy(out=bc, in_=psB)
            o_sb = opool.tile([128, M], F32, tag="o")
            nc.vector.tensor_mul(o_sb, psC, bc)
            odma(out=orr[:, dp0 + dq, mo : mo + M], in_=o_sb)


def _split_multi_waits(nc):
    """This image's walrus accepts at most ONE sem wait per instruction
    (setupSyncWait: 'Too many sync wait commands').  Tile emits joins with
    2+ waits (its transitive-minimality pass is disabled), so hoist all but
    the last wait of every instruction into 1-wait NoOps emitted just
    before it on the same engine stream."""
    nid = [0]

    def noop(engine, wait):
        nid[0] += 1
        return mybir.InstNoOp(
            name=f"waitsplit-{nid[0]}",
            ins=[],
            outs=[],
            engine=engine,
            sync_info=mybir.SyncInfo(on_wait=[wait], on_update=[]),
        )

    for f in nc.m.functions:
        for blk in f.blocks:
            insts = blk.instructions
            out = []
            changed = False
            for inst in insts:
                si = getattr(inst, "sync_info", None)
                waits = list(si.on_wait) if si is not None and si.on_wait else []
                if len(waits) > 1:
                    changed = True
                    for w in waits[:-1]:
                        out.append(noop(inst.engine, w))
                    inst.sync_info = mybir.SyncInfo(
                        on_wait=[waits[-1]], on_update=list(si.on_update)
                    )
                out.append(inst)
            if changed:
                insts[:] = out


def _build_nc():
    nc = bass.Bass("TRN2")
    q1 = nc.dram_tensor("q1", [2048, 1024], F16, kind="ExternalInput")
    k1 = nc.dram_tensor("k1", [32, 128, 16, 128], F16, kind="ExternalInput")
    v1 = nc.dram_tensor("v1", [16, 128, 32, 128], F16, kind="ExternalInput")
    q0 = nc.dram_tensor("q0", [8192, 256], F16, kind="ExternalInput")
    k0 = nc.dram_tensor("k0", [8, 128, 64, 128], F16, kind="ExternalInput")
    v0 = nc.dram_tensor("v0", [64, 128, 8, 128], F16, kind="ExternalInput")
    o1 = nc.dram_tensor("o1", [2048, 1024], F32, kind="ExternalOutput")
    o0 = nc.dram_tensor("o0", [8192, 256], F32, kind="ExternalOutput")

    with tile.TileContext(nc) as tc:
        with (
            tc.tile_pool(name="qpool", bufs=1) as qpool,
            tc.tile_pool(name="epool", bufs=2) as epool,
            tc.tile_pool(name="kpool", bufs=2) as kpool,
            tc.tile_pool(name="vpool", bufs=2) as vpool,
            tc.tile_pool(name="spool", bufs=1) as spool,
            tc.tile_pool(name="opool", bufs=4) as opool,
            tc.tile_pool(name="pa", bufs=3, space="PSUM") as pa,
            tc.tile_pool(name="ps", bufs=1, space="PSUM") as ps,
            tc.tile_pool(name="pb", bufs=1, space="PSUM") as pb,
            tc.tile_pool(name="pc", bufs=3, space="PSUM") as pc,
        ):
            ones_p = spool.tile([128, 1], F16, tag="ones_p")
            nc.vector.memset(ones_p, 1.0)
            ones_1 = spool.tile([1, 128], F32, tag="ones_1")
            nc.vector.memset(ones_1, 1.0)
            pools = (qpool, epool, kpool, vpool, spool, opool, pa, ps, pb, pc, ones_p, ones_1)
            _emit_job(tc, pools, q1, k1, v1, o1, 2048, 4096, 0, 512, vg=1)
            _emit_job(tc, pools, q1, k1, v1, o1, 2048, 4096, 512, 512, vg=1)
            _emit_job(tc, pools, q0, k0, v0, o0, 8192, 1024, 0, 256, vg=8)
    _split_multi_waits(nc)
    return nc


def _tokens(x, ph, pw):
    """(16, 128, 64, 64) -> (N, D) with n=(t,oh,ow), d=(c,ih,iw)."""
    t = x.reshape(16, 128, 64 // ph, ph, 64 // pw, pw)
    t = t.transpose(0, 2, 4, 1, 3, 5)
    return np.ascontiguousarray(t.reshape(16 * (64 // ph) * (64 // pw), 128 * ph * pw))


def _untokens(v, ph, pw):
    """(N, D) -> (16, 128, 64, 64)."""
    oh = 64 // ph
    t = v.reshape(16, oh, oh, 128, ph, pw)
    t = t.transpose(0, 3, 1, 4, 2, 5)
    return t.reshape(16, 128, 64, 64)


def _pack_scale(q, k, v, b, ph, pw, N, D, M, chan_lo):
    """Host-side shard prep for one (batch, scale): returns dict pieces."""
    sl = np.s_[b * 16 : (b + 1) * 16, chan_lo : chan_lo + 128]
    qt = _tokens(q[sl], ph, pw)  # [N, D]
    kt = _tokens(k[sl], ph, pw)
    vt = _tokens(v[sl], ph, pw)
    scale = np.float32(1.0 / np.sqrt(D))
    NS, DT = N // 128, D // 128
    # q^T per m-chunk, pre-scaled:
    qT = np.ascontiguousarray((qt.T * scale).astype(np.float16))  # [D, N]
    kh = np.ascontiguousarray(
        kt.T.reshape(DT, 128, NS, 128).transpose(2, 1, 0, 3).astype(np.float16)
    )  # [NS, 128, DT, 128]
    vh = np.ascontiguousarray(
        vt.reshape(NS, 128, DT, 128).transpose(2, 1, 0, 3).astype(np.float16)
    )  # [DT, 128, NS, 128]
    return qT, kh, vh


def kernel(query, key, value, masks):
    global LAST_EXEC_NS, LAST_RESULTS, _NC_CACHE
    query = np.asarray(query)
    key = np.asarray(key)
    value = np.asarray(value)

    if _NC_CACHE is None:
        _NC_CACHE = _build_nc()
    nc = _NC_CACHE

    in_maps = []
    packs = {}
    for b in range(B):
        packs[(b, 1)] = _pack_scale(query, key, value, b, *SCALE1)
        packs[(b, 0)] = _pack_scale(query, key, value, b, *SCALE0)
    for core in range(8):
        b, mc = core // 4, core % 4
        q1T, k1h, v1h = packs[(b, 1)]
        q0T, k0h, v0h = packs[(b, 0)]
        in_maps.append(
            {
                "q1": np.ascontiguousarray(q1T[:, mc * 1024 : (mc + 1) * 1024]),
                "k1": k1h,
                "v1": v1h,
                "q0": np.ascontiguousarray(q0T[:, mc * 256 : (mc + 1) * 256]),
                "k0": k0h,
                "v0": v0h,
            }
        )

    trace = bool(int(os.environ.get("BASS_ATTN_TRACE", "0")))
    tmpdir = os.environ.get("BASS_ATTN_TMPDIR") or None
    res = run_bass_kernel_spmd(
        nc, in_maps, core_ids=list(range(8)), trace=trace, tmpdir=tmpdir
    )
    LAST_RESULTS = res
    LAST_EXEC_NS = res.exec_time_ns

    out = np.empty((B * T, C, H, W), dtype=np.float32)
    for b in range(B):
        val1 = np.empty((4096, 2048), dtype=np.float32)
        val0 = np.empty((1024, 8192), dtype=np.float32)
        for mc in range(4):
            r = res.results[b * 4 + mc]
            val1[mc * 1024 : (mc + 1) * 1024] = r["o1"].T
            val0[mc * 256 : (mc + 1) * 256] = r["o0"].T
        out[b * 16 : (b + 1) * 16, 0:128] = _untokens(val0, 8, 8)
        out[b * 16 : (b + 1) * 16, 128:256] = _untokens(val1, 4, 4)
    return out


# revision 14
# speedup vs baseline: 1.1572x; 1.0365x over previous
"""Two-scale patch attention (nn_Atten_module_86938728006215) on 8 TRN2 cores.

Sharding: core k handles batch b = k//4, query-row chunk mc = k%4 for BOTH
patch scales (scale1 = 4x4 patches -> N=4096, D=2048, 1024 rows/core;
scale0 = 8x8 patches -> N=1024, D=8192, 256 rows/core).  Every core does an
identical amount of work (~2.15e10 MACs) and needs no collectives.

The _to_tokens permutation is applied on the host during sharding, so every
device DMA is a contiguous stream:
  qT  [D, M]           feature-major query block, pre-scaled by 1/sqrt(D), f16
  kh  [NS, 128, DT, 128]  K^T n-strips:  kh[s, p, dt, n] = K^T[dt*128+p, s*128+n]
  vh  [DT, 128, NS, 128]  V d'-strips:   vh[dp, p, ns, j] = V[ns*128+p, dp*128+j]
Device computes S^T = K Q^T (so softmax's sum is a ones-matmul over the
partition axis), exp on ScalarE (no max subtraction needed: scores ~ N(0,1)),
O^T = V expS^T, then a broadcast-multiply by 1/rowsum.  Output oT [D, M] f32.
"""

import os
import numpy as np

import concourse.bass as bass
import concourse.mybir as mybir
import concourse.tile as tile
from concourse.bass_utils import run_bass_kernel_spmd

F16 = mybir.dt.float16
F32 = mybir.dt.float32
EXP = mybir.ActivationFunctionType.Exp

B, T, C, H, W = 2, 16, 256, 64, 64
# (ph, pw, N, D, M_per_core, chan_lo)
SCALE1 = (4, 4, 4096, 2048, 1024, 128)
SCALE0 = (8, 8, 1024, 8192, 256, 0)

LAST_EXEC_NS = None
LAST_RESULTS = None
_NC_CACHE = None


def _emit_job(tc, pools, qdram, kdram, vdram, odram, D, N, mo, M, vg):
    """One attention job: query columns [mo, mo+M) of a [D, N] problem.

    vg = number of d'-tiles fetched per V DMA (bigger -> fewer, larger DMAs).
    """
    nc = tc.nc
    DT, NS = D // 128, N // 128
    vdma = nc.sync.dma_start
    odma = nc.scalar.dma_start
    qpool, epool, kpool, vpool, spool, opool, pa, ps, pb, pc, ones_p, ones_1 = pools
    tagsfx = f"d{D}"

    qr = qdram[:, :].rearrange("(dt p) m -> p dt m", p=128)
    orr = odram[:, :].rearrange("(dt p) m -> p dt m", p=128)

    def load_k(ns):
        h = DT // 2
        nb = 3 if D == 2048 else 2
        src2 = kdram[ns].rearrange("p a b -> p (a b)")
        ka = kpool.tile([128, h * 128], F16, tag="ka" + tagsfx, bufs=nb)
        nc.sync.dma_start(out=ka, in_=src2[:, : h * 128])
        kb = kpool.tile([128, h * 128], F16, tag="kb" + tagsfx, bufs=nb)
        nc.sync.dma_start(out=kb, in_=src2[:, h * 128 :])
        return (ka, kb)

    def k_slice(parts, dt):
        h = DT // len(parts)
        return parts[dt // h][:, (dt % h) * 128 : (dt % h + 1) * 128]

    kparts0 = load_k(0)
    q_sb = qpool.tile([128, DT, M], F16, tag="q")
    if D == 2048:
        for dt in range(DT):
            nc.sync.dma_start(out=q_sb[:, dt, :], in_=qr[:, dt, mo : mo + M])
    else:
        nc.sync.dma_start(out=q_sb, in_=qr[:, :, mo : mo + M])

    expS = epool.tile([128, NS, M], F16, tag="e")

    # ---- Phase A: S^T strips + exp ----
    for ns in range(NS):
        kparts = kparts0 if ns == 0 else load_k(ns)
        psA = pa.tile([128, M], F32, tag="pa")
        for dt in range(DT):
            nc.tensor.matmul(
                psA,
                k_slice(kparts, dt),
                q_sb[:, dt, :],
                start=(dt == 0),
                stop=(dt == DT - 1),
            )
        nc.scalar.activation(expS[:, ns, :], psA, EXP)

    # ---- softmax denominators: column sums of expS via ones-matmul ----
    psS = ps.tile([1, M], F32, tag="ps")
    for ns in range(NS):
        nc.tensor.matmul(
            psS, ones_p, expS[:, ns, :], start=(ns == 0), stop=(ns == NS - 1)
        )
    recip = spool.tile([1, M], F32, tag="recip", bufs=2)
    nc.vector.reciprocal(recip, psS)

    # ---- Phase C: O^T = V expS^T, normalized ----
    # The 1/rowsum broadcast matmul is emitted after the first C-group's
    # matmuls so the PE never stalls waiting on the DVE reciprocal.
    bc = None
    vr = vdram[:, :, :, :].rearrange("q p n j -> p q (n j)")
    for dp0 in range(0, DT, vg):
        v_sb = vpool.tile([128, vg, NS * 128], F16, tag="v" + tagsfx)
        vdma(out=v_sb, in_=vr[:, dp0 : dp0 + vg, :])
        for dq in range(vg):
            psC = pc.tile([128, M], F32, tag="pc")
            for ns in range(NS):
                nc.tensor.matmul(
                    psC,
                    v_sb[:, dq, ns * 128 : (ns + 1) * 128],
                    expS[:, ns, :],
                    start=(ns == 0),
                    stop=(ns == NS - 1),
                )
            if bc is None:
                psB = pb.tile([128, M], F32, tag="pb")
                nc.tensor.matmul(psB, ones_1, recip, start=True, stop=True)
                bc = spool.tile([128, M], F16, tag="bc", bufs=2)
                nc.vector.tensor_copy(out=bc, in_=psB)
            o_sb = opool.tile([128, M], F32, tag="o")
            nc.vector.tensor_mul(o_sb, psC, bc)
            odma(out=orr[:, dp0 + dq, mo : mo + M], in_=o_sb)


def _split_multi_waits(nc):
    """This image's walrus accepts at most ONE sem wait per instruction
    (setupSyncWait: 'Too many sync wait commands').  Tile emits joins with
    2+ waits (its transitive-minimality pass is disabled), so hoist all but
    the last wait of every instruction into 1-wait NoOps emitted just
    before it on the same engine stream."""
    nid = [0]

    def noop(engine, wait):
        nid[0] += 1
        return mybir.InstNoOp(
            name=f"waitsplit-{nid[0]}",
            ins=[],
            outs=[],
            engine=engine,
            sync_info=mybir.SyncInfo(on_wait=[wait], on_update=[]),
        )

    for f in nc.m.functions:
        for blk in f.blocks:
            insts = blk.instructions
            out = []
            changed = False
            for inst in insts:
                si = getattr(inst, "sync_info", None)
                waits = list(si.on_wait) if si is not None and si.on_wait else []
                if len(waits) > 1:
                    changed = True
                    for w in waits[:-1]:
                        out.append(noop(inst.engine, w))
                    inst.sync_info = mybir.SyncInfo(
                        on_wait=[waits[-1]], on_update=list(si.on_update)
                    )
                out.append(inst)
            if changed:
                insts[:] = out


def _build_nc():
    nc = bass.Bass("TRN2")
    q1 = nc.dram_tensor("q1", [2048, 1024], F16, kind="ExternalInput")
    k1 = nc.dram_tensor("k1", [32, 128, 16, 128], F16, kind="ExternalInput")
    v1 = nc.dram_tensor("v1", [16, 128, 32, 128], F16, kind="ExternalInput")
    q0 = nc.dram_tensor("q0", [8192, 256], F16, kind="ExternalInput")
    k0 = nc.dram_tensor("k0", [8, 128, 64, 128], F16, kind="ExternalInput")
    v0 = nc.dram_tensor("v0", [64, 128, 8, 128], F16, kind="ExternalInput")
    o1 = nc.dram_tensor("o1", [2048, 1024], F32, kind="ExternalOutput")
    o0 = nc.dram_tensor("o0", [8192, 256], F32, kind="ExternalOutput")

    with tile.TileContext(nc) as tc:
        with (
            tc.tile_pool(name="qpool", bufs=1) as qpool,
            tc.tile_pool(name="epool", bufs=2) as epool,
            tc.tile_pool(name="kpool", bufs=2) as kpool,
            tc.tile_pool(name="vpool", bufs=2) as vpool,
            tc.tile_pool(name="spool", bufs=1) as spool,
            tc.tile_pool(name="opool", bufs=4) as opool,
            tc.tile_pool(name="pa", bufs=3, space="PSUM") as pa,
            tc.tile_pool(name="ps", bufs=1, space="PSUM") as ps,
            tc.tile_pool(name="pb", bufs=1, space="PSUM") as pb,
            tc.tile_pool(name="pc", bufs=3, space="PSUM") as pc,
        ):
            ones_p = spool.tile([128, 1], F16, tag="ones_p")
            nc.vector.memset(ones_p, 1.0)
            ones_1 = spool.tile([1, 128], F32, tag="ones_1")
            nc.vector.memset(ones_1, 1.0)
            pools = (qpool, epool, kpool, vpool, spool, opool, pa, ps, pb, pc, ones_p, ones_1)
            _emit_job(tc, pools, q1, k1, v1, o1, 2048, 4096, 0, 512, vg=1)
            _emit_job(tc, pools, q1, k1, v1, o1, 2048, 4096, 512, 512, vg=1)
            _emit_job(tc, pools, q0, k0, v0, o0, 8192, 1024, 0, 256, vg=8)
    _split_multi_waits(nc)
    return nc


def _tokens(x, ph, pw):
    """(16, 128, 64, 64) -> (N, D) with n=(t,oh,ow), d=(c,ih,iw)."""
    t = x.reshape(16, 128, 64 // ph, ph, 64 // pw, pw)
    t = t.transpose(0, 2, 4, 1, 3, 5)
    return np.ascontiguousarray(t.reshape(16 * (64 // ph) * (64 // pw), 128 * ph * pw))


def _untokens(v, ph, pw):
    """(N, D) -> (16, 128, 64, 64)."""
    oh = 64 // ph
    t = v.reshape(16, oh, oh, 128, ph, pw)
    t = t.transpose(0, 3, 1, 4, 2, 5)
    return t.reshape(16, 128, 64, 64)


def _pack_scale(q, k, v, b, ph, pw, N, D, M, chan_lo):
    """Host-side shard prep for one (batch, scale): returns dict pieces."""
    sl = np.s_[b * 16 : (b + 1) * 16, chan_lo : chan_lo + 128]
    qt = _tokens(q[sl], ph, pw)  # [N, D]
    kt = _tokens(k[sl], ph, pw)
    vt = _tokens(v[sl], ph, pw)
    scale = np.float32(1.0 / np.sqrt(D))
    NS, DT = N // 128, D // 128
    # q^T per m-chunk, pre-scaled:
    qT = np.ascontiguousarray((qt.T * scale).astype(np.float16))  # [D, N]
    kh = np.ascontiguousarray(
        kt.T.reshape(DT, 128, NS, 128).transpose(2, 1, 0, 3).astype(np.float16)
    )  # [NS, 128, DT, 128]
    vh = np.ascontiguousarray(
        vt.reshape(NS, 128, DT, 128).transpose(2, 1, 0, 3).astype(np.float16)
    )  # [DT, 128, NS, 128]
    return qT, kh, vh


def kernel(query, key, value, masks):
    global LAST_EXEC_NS, LAST_RESULTS, _NC_CACHE
    query = np.asarray(query)
    key = np.asarray(key)
    value = np.asarray(value)

    if _NC_CACHE is None:
        _NC_CACHE = _build_nc()
    nc = _NC_CACHE

    in_maps = []
    packs = {}
    for b in range(B):
        packs[(b, 1)] = _pack_scale(query, key, value, b, *SCALE1)
        packs[(b, 0)] = _pack_scale(query, key, value, b, *SCALE0)
    for core in range(8):
        b, mc = core // 4, core % 4
        q1T, k1h, v1h = packs[(b, 1)]
        q0T, k0h, v0h = packs[(b, 0)]
        in_maps.append(
            {
                "q1": np.ascontiguousarray(q1T[:, mc * 1024 : (mc + 1) * 1024]),
                "k1": k1h,
                "v1": v1h,
                "q0": np.ascontiguousarray(q0T[:, mc * 256 : (mc + 1) * 256]),
                "k0": k0h,
                "v0": v0h,
            }
        )

    trace = bool(int(os.environ.get("BASS_ATTN_TRACE", "0")))
    tmpdir = os.environ.get("BASS_ATTN_TMPDIR") or None
    res = run_bass_kernel_spmd(
        nc, in_maps, core_ids=list(range(8)), trace=trace, tmpdir=tmpdir
    )
    LAST_RESULTS = res
    LAST_EXEC_NS = res.exec_time_ns

    out = np.empty((B * T, C, H, W), dtype=np.float32)
    for b in range(B):
        val1 = np.empty((4096, 2048), dtype=np.float32)
        val0 = np.empty((1024, 8192), dtype=np.float32)
        for mc in range(4):
            r = res.results[b * 4 + mc]
            val1[mc * 1024 : (mc + 1) * 1024] = r["o1"].T
            val0[mc * 256 : (mc + 1) * 256] = r["o0"].T
        out[b * 16 : (b + 1) * 16, 0:128] = _untokens(val0, 8, 8)
        out[b * 16 : (b + 1) * 16, 128:256] = _untokens(val1, 4, 4)
    return out
